# revision 3
# baseline (speedup 1.0000x reference)
"""AttentiveMatch kernel for Trainium2 — v4 (fp8 DoubleRow, woven pipeline).

Reference math (per batch):
    pn = l2norm(p); qn = l2norm(q)
    w  = -(pn @ qn^T) / D
    mv = (w @ q) / S
    out = -mean(pn * l2norm(mv), -1)

Device pipeline (scalars folded, sign flips cancel; C=16, SS=HSC=256):
    G^T[j,i]  = q.p                    mm1: fp8 DoubleRow, fp32 PSUM
    ssq_q     = diag(q-gram)           PE block-gram + amr(gram, ident)
    at        = C*rq_j*G       (fp8)   per-partition scaled PSUM drain
    h2        = (G/HSC)*at     (fp8)   DVE affine_mul_reduce out
    dotrow    = ones^T h2              PE DoubleRow reduce -> PSUM row 0
    M^T[d,i]  = sum_j at q             mm2: fp8 DoubleRow
    s2        = (M/SS)^2       (fp8)   ACT square w/ scale
    ssrow     = ones^T s2              PE DoubleRow reduce -> PSUM row 0
    ssq_p     = diag(p-gram)
    out_i     = dotrow / (sp*sqrt(ssrow)),  sp = SS*D/HSC*|p_i|

Rows are transposed to columns by bouncing through a DRAM scratch.  The
batch stages are software-pipelined two deep with a hand-woven per-engine
emission order; the last batch computes its finals in row space to skip
the bounce latency on the critical tail.
"""

import os
import sys

for _p in ("/opt/trn_rl_repo",):
    if _p not in sys.path:
        sys.path.append(_p)

import numpy as np
import ml_dtypes

import concourse.bacc as bacc
import concourse.mybir as mybir
import concourse.tile as tile
from concourse.bass_utils import run_bass_kernel_spmd

B, S, D = 64, 512, 768
NCORES = 8
BP = B // NCORES          # batches per core
ST = S // 128             # s tiles (4)
KT = D // 128             # d subtiles (6)
KP = KT // 2              # double-row k pairs (3)
JPAIRS = ST // 2          # double-row j pairs (2)
F32 = mybir.dt.float32
BF16 = mybir.dt.bfloat16
F8 = mybir.dt.float8e4
AF = mybir.ActivationFunctionType
ALU = mybir.AluOpType
DR = mybir.MatmulPerfMode.DoubleRow

C_AT = 16.0                 # at = C_AT * rq_j * G
SS = 256.0                  # s2 = (M/SS)^2
HSC = 256.0                 # h2 = (G/HSC)*at
K_SP = (SS * D / HSC) ** 2  # sp = sqrt(ssq_p*K_SP)

_NC = None


def _build():
    nc = bacc.Bacc("TRN2", target_bir_lowering=False, debug=False, num_devices=NCORES)
    qt_d = nc.dram_tensor("qt", [BP, 128, KT, S], F8, kind="ExternalInput")
    pt_d = nc.dram_tensor("pt", [BP, 128, KT, S], F8, kind="ExternalInput")
    qn_d = nc.dram_tensor("qn", [BP, 128, ST, D], F8, kind="ExternalInput")
    id_d = nc.dram_tensor("idm", [128, 128], BF16, kind="ExternalInput")
    out_d = nc.dram_tensor("out", [128, BP * ST], F32, kind="ExternalOutput")
    out2_d = nc.dram_tensor("out2", [1, S], F32, kind="ExternalOutput")
    scr_d = nc.dram_tensor("rowscr", [BP, 2, S], F32, kind="Internal")

    with tile.TileContext(nc) as tc:
        with (
            tc.tile_pool(name="cst", bufs=1) as cst,
            tc.tile_pool(name="inp", bufs=4) as inp,
            tc.tile_pool(name="ats", bufs=3) as ats,
            tc.tile_pool(name="gps", bufs=3, space="PSUM") as gps,
            tc.tile_pool(name="mts", bufs=3, space="PSUM") as mts,
            tc.tile_pool(name="grm", bufs=2, space="PSUM") as grm,
            tc.tile_pool(name="st", bufs=3) as st,
            tc.tile_pool(name="jnk", bufs=1) as jnk,
            tc.tile_pool(name="res", bufs=1) as res,
        ):
            # get batch 0's inputs moving before anything else on the rings;
            # qt in two pieces so the first gram blocks can start sooner
            qpt0 = inp.tile([128, 2, KT, S], F8, tag="qpt", name="qpt0")
            nc.sync.dma_start(qpt0[:, 0], qt_d[0])
            nc.gpsimd.dma_start(qpt0[:, 1], pt_d[0])
            qn0 = inp.tile([128, ST, D], F8, tag="qn", name="qn0")
            nc.gpsimd.dma_start(qn0[:], qn_d[0])
            ident = cst.tile([128, 128], BF16)
            nc.sync.dma_start(ident[:], id_d[:, :])
            # DoubleRow ldweights requires >=16 weight columns; row 0 is read
            ones8 = cst.tile([128, 2, 16], F8)
            nc.vector.memset(ones8[:], 1.0)
            wd = res.tile([128, BP * ST], F32)
            junkD = jnk.tile([128, 512], BF16)
            jacc = jnk.tile([128, 1], F32)

            ctx = {}

            def diag(g_t, acc):
                nc.vector.affine_mul_reduce(
                    out=junkD[:, 0:128], accum_out=acc,
                    in0=g_t[:], in1=ident[:], scale=1.0, bias=0.0,
                )

            def gram(src, t, nm, b):
                g_t = grm.tile([128, 128], F32, tag="grm", name=f"{nm}{t}_{b}")
                sl = slice(t * 128, (t + 1) * 128)
                for k in range(KP):
                    nc.tensor.matmul(
                        g_t[:],
                        lhsT=src[:, 2 * k:2 * k + 2, sl],
                        rhs=src[:, 2 * k:2 * k + 2, sl],
                        start=(k == 0), stop=(k == KP - 1), perf_mode=DR,
                    )
                return g_t

            def mm1j(c, j, b):
                g = gps.tile([128, S], F32, tag="g", name=f"g{j}_{b}")
                qt, pt = c["qt"], c["pt"]
                for k in range(KP):
                    nc.tensor.matmul(
                        g[:],
                        lhsT=qt[:, 2 * k:2 * k + 2, j * 128:(j + 1) * 128],
                        rhs=pt[:, 2 * k:2 * k + 2, :],
                        start=(k == 0), stop=(k == KP - 1), perf_mode=DR,
                    )
                c["g"].append(g)

            for it in range(BP + 2):
                b = it          # early batch
                a = it - 1      # late batch
                f = it - 2      # finish batch

                # ---- input DMAs for b
                if 0 < b < BP:
                    c = ctx[b] = {}
                    qpt = inp.tile([128, 2, KT, S], F8, tag="qpt", name=f"qpt{b}")
                    nc.sync.dma_start(qpt[:, 0], qt_d[b])
                    nc.gpsimd.dma_start(qpt[:, 1], pt_d[b])
                    qn = inp.tile([128, ST, D], F8, tag="qn", name=f"qn{b}")
                    nc.gpsimd.dma_start(qn[:], qn_d[b])
                    c["qt"], c["pt"], c["qn"] = qpt[:, 0], qpt[:, 1], qn
                elif b == 0:
                    c = ctx[0] = {}
                    c["qt"], c["pt"], c["qn"] = qpt0[:, 0], qpt0[:, 1], qn0
                if b < BP:
                    cb = ctx[b]
                    cb["g"] = []
                    cb["ssq_q"] = st.tile([128, ST], F32, tag="ssq_q", name=f"sq{b}")
                    cb["ssq_p"] = st.tile([128, ST], F32, tag="ssq_p", name=f"sp{b}")
                ca = ctx.get(a)

                # ---- early(b): grams, diags, mm1, rq chain
                if b < BP:
                    qg = [gram(cb["qt"], 0, "qg", b), gram(cb["qt"], 1, "qg", b)]
                    diag(qg[0], cb["ssq_q"][:, 0:1])
                    diag(qg[1], cb["ssq_q"][:, 1:2])
                    mm1j(cb, 0, b)
                    qg.append(gram(cb["qt"], 2, "qg", b))
                    diag(qg[2], cb["ssq_q"][:, 2:3])
                    mm1j(cb, 1, b)
                    qg.append(gram(cb["qt"], 3, "qg", b))
                    diag(qg[3], cb["ssq_q"][:, 3:4])
                    mm1j(cb, 2, b)
                    mm1j(cb, 3, b)
                    s1 = st.tile([128, ST], F32, tag="s1", name=f"s1{b}")
                    nc.scalar.activation(s1[:], cb["ssq_q"][:], AF.Sqrt,
                                         scale=1.0 / (C_AT * C_AT))
                    crq = st.tile([128, ST], F32, tag="crq", name=f"crq{b}")
                    nc.vector.reciprocal(crq[:], s1[:])
                    cb["crq"] = crq
                    for t in range(ST):
                        pgt = gram(cb["pt"], t, "pg", b)
                        diag(pgt, cb["ssq_p"][:, t:t + 1])

                # ---- late(a): drains of G, reduces, mm2
                if ca is not None:
                    at = ats.tile([128, ST, S], F8, tag="at", name=f"at{a}")
                    h2 = ats.tile([128, ST, S], F8, tag="h2", name=f"h2{a}")
                    for j in range(ST):
                        if j == 0:
                            nc.vector.tensor_scalar_mul(at[:, j, :], ca["g"][j][:],
                                                        ca["crq"][:, j:j + 1])
                        else:
                            nc.scalar.activation(at[:, j, :], ca["g"][j][:], AF.Copy,
                                                 scale=ca["crq"][:, j:j + 1])
                        nc.vector.affine_mul_reduce(
                            out=h2[:, j, :], accum_out=jacc[:], in0=ca["g"][j][:],
                            in1=at[:, j, :], scale=1.0 / HSC, bias=0.0)
                    qn = ca["qn"]
                    last = a == BP - 1
                    # dot row (DoubleRow, dst partitions 0:16; row 0 read)
                    dotrow = mts.tile([128, S], F32, tag="mt", name=f"dot{a}")
                    for u in range(ST // 2):
                        nc.tensor.matmul(
                            dotrow[0:16, :], lhsT=ones8[:],
                            rhs=h2[:, 2 * u:2 * u + 2, :],
                            start=(u == 0), stop=(u == ST // 2 - 1), perf_mode=DR)
                    dsb = st.tile([1, S], F32, tag="dsb", name=f"dsb{a}")
                    nc.vector.tensor_copy(dsb[0:1, :], dotrow[0:1, :])
                    nc.sync.dma_start(scr_d[a, 0], dsb[0:1, :])

                    # mm2 + fused s2 drains
                    s2 = ats.tile([128, KT, S], F8, tag="s2", name=f"s2{a}")
                    for k in range(KT):
                        mt = mts.tile([128, S], F32, tag="mt", name=f"mt{k}_{a}")
                        for jp in range(JPAIRS):
                            nc.tensor.matmul(
                                mt[:],
                                lhsT=qn[:, 2 * jp:2 * jp + 2, k * 128:(k + 1) * 128],
                                rhs=at[:, 2 * jp:2 * jp + 2, :],
                                start=(jp == 0), stop=(jp == JPAIRS - 1),
                                perf_mode=DR)
                        nc.scalar.activation(s2[:, k, :], mt[:], AF.Square,
                                             scale=1.0 / SS)

                    # ss row (DoubleRow, dst partitions 0:16)
                    ssrow = mts.tile([128, S], F32, tag="mt", name=f"ss{a}")
                    for u in range(KT // 2):
                        nc.tensor.matmul(
                            ssrow[0:16, :], lhsT=ones8[:],
                            rhs=s2[:, 2 * u:2 * u + 2, :],
                            start=(u == 0), stop=(u == KT // 2 - 1), perf_mode=DR)
                    ssb = st.tile([1, S], F32, tag="ssb", name=f"ssb{a}")
                    nc.vector.tensor_copy(ssb[0:1, :], ssrow[0:1, :])
                    nc.sync.dma_start(scr_d[a, 1], ssb[0:1, :])
                    dsc = st.tile([128, 2, ST], F32, tag="dsc", name=f"dsc{a}")
                    nc.sync.dma_start(
                        dsc[:, :, :],
                        scr_d[a].rearrange("r (t p) -> p r t", p=128))
                    ca["dsc"] = dsc

                # ---- finals for f (columns path)
                if 0 <= f < BP:
                    cf = ctx.pop(f)
                    dsc = cf["dsc"]
                    sp = st.tile([128, ST], F32, tag="sp", name=f"spf{f}")
                    nc.scalar.activation(sp[:], cf["ssq_p"][:], AF.Sqrt, scale=K_SP)
                    rp = st.tile([128, ST], F32, tag="rp", name=f"rp{f}")
                    nc.vector.reciprocal(rp[:], sp[:])
                    t2 = st.tile([128, ST], F32, tag="t2", name=f"t2{f}")
                    nc.scalar.activation(t2[:], dsc[:, 1, :], AF.Sqrt)
                    r2 = st.tile([128, ST], F32, tag="r2", name=f"r2{f}")
                    nc.vector.reciprocal(r2[:], t2[:])
                    w1 = st.tile([128, ST], F32, tag="w1", name=f"w1{f}")
                    nc.vector.tensor_mul(w1[:], dsc[:, 0, :], rp[:])
                    nc.vector.tensor_mul(wd[:, f * ST:(f + 1) * ST], w1[:], r2[:])
                    nc.sync.dma_start(out_d[:, f * ST:(f + 1) * ST],
                                      wd[:, f * ST:(f + 1) * ST])
    nc.compile()
    return nc


def _get_nc():
    global _NC
    if _NC is None:
        _NC = _build()
    return _NC


def _prep_inputs(p, q):
    p = np.asarray(p, dtype=np.float32)
    q = np.asarray(q, dtype=np.float32)
    p8 = p.astype(ml_dtypes.float8_e4m3)
    q8 = q.astype(ml_dtypes.float8_e4m3)

    # transposed: [core, b, part, k, s] with d = k*128 + part
    def tr(x):
        return np.ascontiguousarray(
            x.reshape(NCORES, BP, S, KT, 128).transpose(0, 1, 4, 3, 2)
        )

    # natural: [core, b, part, t, d] with s = t*128 + part
    def nat(x):
        return np.ascontiguousarray(
            x.reshape(NCORES, BP, ST, 128, D).transpose(0, 1, 3, 2, 4)
        )

    qtr, ptr, qna = tr(q8), tr(p8), nat(q8)
    idm = np.ascontiguousarray(np.eye(128, dtype=ml_dtypes.bfloat16))
    return [
        {"qt": qtr[c], "pt": ptr[c], "qn": qna[c], "idm": idm}
        for c in range(NCORES)
    ]


def _postprocess(results):
    o = np.stack([np.asarray(r["out"], dtype=np.float32) for r in results])
    # o[c, part, b*ST + t] is out for batch c*BP+b at i = t*128 + part
    o = o.reshape(NCORES, 128, BP, ST).transpose(0, 2, 3, 1)
    # last batch per core arrives as a row [S] with i = t*128 + p
    return np.ascontiguousarray(o.reshape(B, 1, S))


def _run(inputs, trace=False, **kw):
    nc = _get_nc()
    in_maps = _prep_inputs(inputs["p"], inputs["q"])
    res = run_bass_kernel_spmd(nc, in_maps, list(range(NCORES)), trace=trace, **kw)
    return _postprocess(res.results), res


def kernel(p, q):
    out, _ = _run({"p": p, "q": q})
    return out
